# revision 35
# baseline (speedup 1.0000x reference)
"""ConcatAttention Trainium2 kernel.

attention(b,:) = sum_s p(b,s) * enc(b,s,:) ;  p = softmax_s(v . tanh(W_enc e + W_dec d))

Sharding: batch (16) split across 8 cores, 2 batches/core, no collectives.

Host-side prep (inside kernel()):
  - per-core encoder shard pre-transposed to eT (2, D, S) so the contraction
    dim (D) lands on SBUF partitions with perfect DMA (no on-chip transpose)
  - dec_proj = decoder_hidden @ W_dec.T  (tiny), W_encT, v replicated
  - encoder mask pre-converted to additive f32 (-1e30 pad / -60 offset)
  - final normalization (divide by Z) + decoder-mask zeroing on host

On-chip per batch, streaming over 8 s-tiles of 512 (no full-batch residency):
  proj psum(h,s) = sum_k W_encT[k] . eT[k]        (fp32r matmuls, N=512)
  X = tanh(proj + dec_proj[b])                     (ACT, bias per partition)
  scores psum(1,s) = sum_j v_j . X_j               (fp32r matvec, M=1)
  U = exp(scores + maskf)                          (ACT; maskf has -60 offset)
  U_bcast psum = ones^T . U                        (PE outer product)
  acc[:,k,t] = sum_s eT[k] * U_bcast               (DVE affine_mul_reduce)
Outputs: U (2,S) unnormalized masked exp, attnU (2,D) unnormalized attention.
"""

import numpy as np

import concourse.bass as bass
import concourse.mybir as mybir
import concourse.tile as tile
from concourse import bacc
from concourse.bass_utils import run_bass_kernel_spmd

B, S, H = 16, 4096, 512
D = 2 * H
NCORES = 8
BPC = B // NCORES          # batches per core
ST = 512                   # s-tile width
NT = S // ST               # s-tiles per batch
KC = D // 128              # contraction chunks (8)
HC = H // 128              # h chunks (4)

F32 = mybir.dt.float32
F32R = mybir.dt.float32r
AF = mybir.ActivationFunctionType
ALU = mybir.AluOpType

# X/U dtype for the tanh/exp outputs that feed PE matmuls.
X_DTYPE = F32R

_prog_cache = {}
last_result = None  # BassKernelResults from the most recent kernel() call


def _build_program():
    nc = bacc.Bacc("TRN2", target_bir_lowering=False, debug=False,
                   num_devices=NCORES)

    eT = nc.dram_tensor("eT", [BPC, D, S], F32R, kind="ExternalInput")
    wencT = nc.dram_tensor("wencT", [D, H], F32R, kind="ExternalInput")
    vh = nc.dram_tensor("vh", [H], F32R, kind="ExternalInput")
    dp = nc.dram_tensor("dp", [BPC, H], F32, kind="ExternalInput")
    maskf = nc.dram_tensor("maskf", [BPC, S], F32, kind="ExternalInput")
    Uo = nc.dram_tensor("U", [BPC, S], F32, kind="ExternalOutput")
    attnU = nc.dram_tensor("attnU", [BPC, D], F32, kind="ExternalOutput")

    with tile.TileContext(nc) as tc:
        with (
            tc.tile_pool(name="const", bufs=1) as const,
            tc.tile_pool(name="epool", bufs=6) as epool,
            tc.tile_pool(name="xpool", bufs=3) as xpool,
            tc.tile_pool(name="upool", bufs=4) as upool,
            tc.tile_pool(name="spool", bufs=2) as spool,
            tc.tile_pool(name="accpool", bufs=2) as accpool,
            tc.tile_pool(name="ps_proj", bufs=5, space="PSUM") as ps_proj,
            tc.tile_pool(name="ps_s", bufs=1, space="PSUM") as ps_s,
            tc.tile_pool(name="ps_bc", bufs=2, space="PSUM") as ps_bc,
        ):
            # W load split per contraction chunk so the first proj matmuls
            # can start as soon as w[k=0] (256 KiB) lands.
            w_sb = [const.tile([128, H], F32R, name=f"w{k}", tag=f"w{k}")
                    for k in range(KC)]
            wv = wencT.ap().rearrange("(c p) x -> c p x", p=128)
            v_sb = const.tile([128, HC], F32R)
            for k in range(KC):
                nc.gpsimd.dma_start(out=w_sb[k], in_=wv[k])
            nc.gpsimd.dma_start(
                out=v_sb, in_=vh.ap().rearrange("(c p) -> p c", p=128))
            dp_sb = const.tile([128, BPC, HC], F32)
            for b in range(BPC):
                nc.gpsimd.dma_start(
                    out=dp_sb[:, b, :],
                    in_=dp.ap()[b].rearrange("(c p) -> p c", p=128))
            dummy = const.tile([128, 1], F32)
            ones_f32 = const.tile([1, 128], F32)
            nc.vector.memset(ones_f32, 1.0)
            ones_sb = const.tile([1, 128], F32R)
            nc.vector.tensor_copy(ones_sb, ones_f32)

            accs = [accpool.tile([128, KC, NT], F32, name=f"acc{b}",
                                 tag=f"acc{b}")
                    for b in range(BPC)]
            # batches interleaved: independent per-batch chains fill each
            # other's cross-engine stalls (and overlap the tail drain)
            for t in range(NT):
                for b in range(BPC):
                    acc = accs[b]
                    ssl = slice(t * ST, (t + 1) * ST)
                    # quarter-loads (512 KiB each) for fine-grained pipelining
                    ev = eT.ap()[b, :, ssl].rearrange(
                        "(q c p) s -> q p c s", q=4, p=128)
                    et = [epool.tile([128, KC // 4, ST], F32R, name=f"et{q}",
                                     tag=f"et{q}")
                          for q in range(4)]
                    for q in range(4):
                        nc.sync.dma_start(out=et[q], in_=ev[q])
                    mt = spool.tile([1, ST], F32, tag="mt")
                    nc.gpsimd.dma_start(out=mt, in_=maskf.ap()[b:b + 1, ssl])

                    X = xpool.tile([128, HC, ST], X_DTYPE, tag="X")
                    for j in range(HC):
                        pp = ps_proj.tile([128, ST], F32, tag="proj")
                        hsl = slice(j * 128, (j + 1) * 128)
                        for k in range(KC):
                            nc.tensor.matmul(
                                pp,
                                lhsT=w_sb[k][:, hsl],
                                rhs=et[k // 2][:, k % 2, :],
                                start=(k == 0), stop=(k == KC - 1))
                        nc.scalar.activation(
                            out=X[:, j, :], in_=pp, func=AF.Tanh,
                            bias=dp_sb[:, b, j:j + 1], scale=1.0)

                    sp = ps_s.tile([1, ST], F32, tag="sc")
                    for j in range(HC):
                        nc.tensor.matmul(
                            sp, lhsT=v_sb[:, j:j + 1], rhs=X[:, j, :],
                            start=(j == 0), stop=(j == HC - 1))
                    sm = spool.tile([1, ST], F32, tag="sm")
                    nc.vector.tensor_tensor(sm, sp, mt, ALU.add)
                    U_sb = upool.tile([1, ST], F32R, tag="Ut")
                    nc.scalar.activation(out=U_sb, in_=sm, func=AF.Exp)

                    nc.gpsimd.dma_start(out=Uo.ap()[b:b + 1, ssl],
                                        in_=U_sb.bitcast(F32))
                    ub = ps_bc.tile([128, ST], F32, tag="ub")
                    nc.tensor.matmul(ub, lhsT=ones_sb, rhs=U_sb,
                                     start=True, stop=True)
                    for k in range(KC):
                        nc.vector.affine_mul_reduce(
                            out=dummy.broadcast_to((128, ST)),
                            accum_out=acc[:, k, t:t + 1],
                            in0=et[k // 2][:, k % 2, :].bitcast(F32), in1=ub,
                            scale=1.0, bias=0.0)

            for b in range(BPC):
                asum = accpool.tile([128, KC], F32, tag="asum", bufs=2)
                nc.vector.tensor_reduce(
                    asum, accs[b], axis=mybir.AxisListType.X, op=ALU.add)
                nc.gpsimd.dma_start(
                    out=attnU.ap()[b].rearrange("(c p) -> p c", p=128),
                    in_=asum)

    nc.compile()
    return nc


def _get_program():
    if "nc" not in _prog_cache:
        _prog_cache["nc"] = _build_program()
    return _prog_cache["nc"]


def kernel(encoder_hidden, encoder_mask, decoder_hidden, decoder_mask,
           W_a, v_a, **_unused):
    encoder_hidden = np.asarray(encoder_hidden, dtype=np.float32)
    encoder_mask = np.asarray(encoder_mask)
    decoder_hidden = np.asarray(decoder_hidden, dtype=np.float32)
    decoder_mask = np.asarray(decoder_mask)
    W_a = np.asarray(W_a, dtype=np.float32)
    v_a = np.asarray(v_a, dtype=np.float32)

    d = encoder_hidden.shape[-1]
    wencT = np.ascontiguousarray(W_a[:, :d].T)             # (D, H)
    dec_proj = decoder_hidden @ W_a[:, d:].T               # (B, H) fp32
    maskf = np.where(encoder_mask, np.float32(-1e30),
                     np.float32(-60.0)).astype(np.float32)  # additive + offset

    in_maps = []
    for c in range(NCORES):
        sl = slice(c * BPC, (c + 1) * BPC)
        in_maps.append({
            "eT": np.ascontiguousarray(
                encoder_hidden[sl].transpose(0, 2, 1)),    # (BPC, D, S)
            "wencT": wencT,
            "vh": v_a,
            "dp": np.ascontiguousarray(dec_proj[sl]),
            "maskf": np.ascontiguousarray(maskf[sl]),
        })

    nc = _get_program()
    res = run_bass_kernel_spmd(nc, in_maps, core_ids=list(range(NCORES)))
    global last_result
    last_result = res

    U = np.concatenate([r["U"] for r in res.results], axis=0)        # (B, S)
    aU = np.concatenate([r["attnU"] for r in res.results], axis=0)   # (B, D)
    Z = U.astype(np.float64).sum(axis=1, keepdims=True)
    distribution = (U / Z).astype(np.float32)
    attention = (aU / Z).astype(np.float32)
    attention[decoder_mask] = 0.0
    return attention, distribution
